# revision 1
# baseline (speedup 1.0000x reference)
"""VQ-codebook + tiny attention + FC kernel for TRN2 (8 NeuronCores, SPMD).

Problem: nn_CodeBook (vq_codebook). For each of 16384 sub-tokens (64-dim),
find the nearest (cosine) codebook row among 16384, substitute the raw row,
run a 2-token attention mix and a fused FC + QuickGELU.

Strategy (data-parallel over batch, 128 batches/core):
  Launch A (device): bf16 screen matmul t_raw @ c_n^T (argmax is invariant to
    the positive row scale, so tokens are not normalized), ACT drains psum to
    an fp16 score copy, DVE folds 16384 -> 8192 -> 4096 "quad maxes",
    max8 + max_index give the top-8 quad positions per row (duplicate values
    advance to distinct positions). Top-4 quads x 4 members = 16 exact
    candidate indices per row.
  Host: expands quad positions to candidate codebook indices and gathers the
    raw codebook vectors (pure indexing / layout prep, no arithmetic), and
    pre-transposes the weight matrices for the device's K-major matmuls.
  Launch B (device): exact fp32 rescore of the 16 candidates per row picks
    the winner (zero argmin flips vs the fp32 reference; measured min top-2
    gap of this problem is 1.4e-6 while the rescore error is ~1e-7), builds
    new_tok, then computes the attention mix + FC in fp32:
      scores_ts[b] = sum_d (K@fuse_t^T)[d,b] * (Q@fuse_s^T)[d,b]
      A = softmax_t(scores/sqrt(2));  mix_s = A_0s*cls + A_1s*new
      h = [V@mix_0^T ; V@mix_1^T]^T @ Wf^T + bf;  h *= sigmoid(1.702 h)

The GPSIMD custom-ucode gather ops (dma_gather / ap_gather / indirect DMA)
are unavailable on this image (BEDROCK=1), hence the host gather between the
two launches.
"""
import os
import sys
import numpy as np
from contextlib import ExitStack

for _p in ("/opt/trn_rl_repo", "/root/.axon_site/_ro/trn_rl_repo"):
    if os.path.isdir(_p) and _p not in sys.path:
        sys.path.append(_p)

import concourse.bass as bass
import concourse.bacc as bacc
import concourse.tile as tile
from concourse import mybir
from concourse.bass_utils import run_bass_kernel_spmd

FP32 = mybir.dt.float32
FP16 = mybir.dt.float16
BF16 = mybir.dt.bfloat16
U32 = mybir.dt.uint32

P = 128          # partitions / batches per core
DIM = 1024
CD = 64          # code dim
BOOK = 16384
NCORES = 8
NT = DIM // CD   # 16 t-chunks (sub-token groups) per core
NCH = BOOK // 512  # 32 matmul chunks per t-chunk
NQ = 4           # top quads kept
LC = 4 * NQ      # 16 candidates per row

_cache = {}
_PROFILE_DIR = None   # set by test harness to capture NTFF profiles per launch


def _build_a():
    nc = bacc.Bacc("TRN2", debug=False)
    cls_d = nc.declare_dram_parameter("cls", [P, DIM], FP32, isOutput=False)
    cb_d = nc.declare_dram_parameter("cb", [BOOK, CD], FP32, isOutput=False)
    idn_d = nc.declare_dram_parameter("idn", [P, P], BF16, isOutput=False)
    idx_d = nc.declare_dram_parameter("idx", [P, NT * 8], U32, isOutput=True)

    with ExitStack() as ctx:
        tc = ctx.enter_context(tile.TileContext(nc))
        const = ctx.enter_context(tc.tile_pool(name="const", bufs=1))
        main = ctx.enter_context(tc.tile_pool(name="main", bufs=1))

        idn = const.tile([P, P], BF16)
        nc.sync.dma_start(idn[:], idn_d[:])

        cls_f = main.tile([P, DIM], FP32)
        nc.sync.dma_start(cls_f[:], cls_d[:])
        cls_b = main.tile([P, DIM], BF16)
        nc.vector.tensor_copy(cls_b[:], cls_f[:])

        cT = main.tile([CD, BOOK], BF16)    # normalized codebook, K-major
        tT = main.tile([CD, NT * P], BF16)  # raw tokens, K-major

        with tc.tile_pool(name="prep", bufs=1) as prep, \
                tc.tile_pool(name="pst", bufs=4, space="PSUM") as pst:
            cb_nat = prep.tile([P, (BOOK // P) * CD], FP32)   # (128, 8192)
            nc.sync.dma_start(
                cb_nat[:], cb_d[:].rearrange("(c p) k -> p c k", p=P))
            sq = prep.tile([P, (BOOK // P) * CD], FP32)
            nc.vector.tensor_tensor(out=sq[:], in0=cb_nat[:], in1=cb_nat[:],
                                    op=mybir.AluOpType.mult)
            csq = prep.tile([P, BOOK // P], FP32)
            nc.vector.tensor_reduce(
                out=csq[:], in_=sq[:].rearrange("p (c k) -> p c k", k=CD),
                axis=mybir.AxisListType.X, op=mybir.AluOpType.add)
            rec = prep.tile([P, BOOK // P], FP32)
            nc.vector.reciprocal(rec[:], csq[:])
            inv = prep.tile([P, BOOK // P], FP32)
            nc.scalar.activation(inv[:], rec[:],
                                 mybir.ActivationFunctionType.Sqrt)
            c_nb = prep.tile([P, (BOOK // P) * CD], BF16)
            nc.vector.tensor_tensor(
                out=c_nb[:].rearrange("p (c k) -> p c k", k=CD),
                in0=cb_nat[:].rearrange("p (c k) -> p c k", k=CD),
                in1=inv[:].rearrange("p (c o) -> p c o", o=1)
                         .broadcast_to([P, BOOK // P, CD]),
                op=mybir.AluOpType.mult)
            # transpose the normalized codebook: 128 x (128, 64) -> (64, 128)
            for c in range(BOOK // P):
                tp = pst.tile([CD, P], BF16, tag="tp")
                nc.tensor.transpose(tp[:], c_nb[:, c * CD:(c + 1) * CD],
                                    idn[:])
                nc.scalar.copy(cT[:, c * P:(c + 1) * P], tp[:])
            # transpose raw tokens: 16 x (128, 64) -> (64, 128)
            for g in range(NT):
                tp = pst.tile([CD, P], BF16, tag="tp")
                nc.tensor.transpose(tp[:], cls_b[:, g * CD:(g + 1) * CD],
                                    idn[:])
                nc.scalar.copy(tT[:, g * P:(g + 1) * P], tp[:])

        spool = ctx.enter_context(tc.tile_pool(name="spool", bufs=2))
        hpool = ctx.enter_context(tc.tile_pool(name="hpool", bufs=2))
        small = ctx.enter_context(tc.tile_pool(name="small", bufs=4))
        psmm = ctx.enter_context(tc.tile_pool(name="psmm", bufs=2, space="PSUM"))

        idx_t = main.tile([P, NT * 8], U32)

        for m in range(NT):
            S = spool.tile([P, BOOK], FP16, tag="S")
            for grp in range(8):
                ps = psmm.tile([P, 2048], FP32, tag="mm")
                for q in range(4):
                    n = 4 * grp + q
                    nc.tensor.matmul(
                        ps[:, q * 512:(q + 1) * 512],
                        lhsT=tT[:, m * P:(m + 1) * P],
                        rhs=cT[:, n * 512:(n + 1) * 512],
                        start=True, stop=True)
                nc.scalar.copy(S[:, grp * 2048:(grp + 1) * 2048], ps[:])
            H = hpool.tile([P, BOOK // 2], FP16, tag="H")
            nc.vector.tensor_tensor(out=H[:], in0=S[:, :BOOK // 2],
                                    in1=S[:, BOOK // 2:],
                                    op=mybir.AluOpType.max)
            H2 = hpool.tile([P, BOOK // 4], FP16, tag="H2")
            nc.vector.tensor_tensor(out=H2[:], in0=H[:, :BOOK // 4],
                                    in1=H[:, BOOK // 4:],
                                    op=mybir.AluOpType.max)
            pv = small.tile([P, 8], FP16, tag="pv")
            nc.vector.max(pv[:], H2[:])
            pu = small.tile([P, 8], U32, tag="pu")
            nc.vector.max_index(pu[:], pv[:], H2[:])
            nc.vector.tensor_copy(idx_t[:, m * 8:(m + 1) * 8], pu[:])
        nc.sync.dma_start(idx_d[:], idx_t[:])
    nc.compile()
    return nc


def _build_b():
    nc = bacc.Bacc("TRN2", debug=False)
    cls_d = nc.declare_dram_parameter("cls", [P, DIM], FP32, isOutput=False)
    clsT_d = nc.declare_dram_parameter("clsT", [DIM, P], FP32, isOutput=False)
    cand_d = nc.declare_dram_parameter("cand", [P, NT * CD * LC], FP32,
                                       isOutput=False)
    iota_d = nc.declare_dram_parameter("iota", [P, LC], FP32, isOutput=False)
    idnf_d = nc.declare_dram_parameter("idnf", [P, P], FP32, isOutput=False)
    kt_d = nc.declare_dram_parameter("KT", [DIM, DIM], FP32, isOutput=False)
    qt_d = nc.declare_dram_parameter("QT", [DIM, DIM], FP32, isOutput=False)
    vt_d = nc.declare_dram_parameter("VT", [DIM, DIM], FP32, isOutput=False)
    wft_d = nc.declare_dram_parameter("WfT", [2 * DIM, DIM], FP32,
                                      isOutput=False)
    bf_d = nc.declare_dram_parameter("bfv", [1, DIM], FP32, isOutput=False)
    h_d = nc.declare_dram_parameter("h", [P, DIM], FP32, isOutput=True)

    EB = DIM // P  # 8 e-blocks

    with ExitStack() as ctx:
        tc = ctx.enter_context(tile.TileContext(nc))
        const = ctx.enter_context(tc.tile_pool(name="const", bufs=1))
        main = ctx.enter_context(tc.tile_pool(name="main", bufs=1))
        pst = ctx.enter_context(tc.tile_pool(name="pst", bufs=1, space="PSUM"))
        pskq = ctx.enter_context(tc.tile_pool(name="pskq", bufs=1, space="PSUM"))
        psh = ctx.enter_context(tc.tile_pool(name="psh", bufs=2, space="PSUM"))
        scr = ctx.enter_context(tc.tile_pool(name="scr", bufs=2))
        rscr = ctx.enter_context(tc.tile_pool(name="rscr", bufs=2))
        p2 = ctx.enter_context(tc.tile_pool(name="p2", bufs=1))
        p2s = ctx.enter_context(tc.tile_pool(name="p2s", bufs=1))
        wpool = ctx.enter_context(tc.tile_pool(name="wpool", bufs=1))
        vpool = ctx.enter_context(tc.tile_pool(name="vpool", bufs=1))

        idn = const.tile([P, P], FP32)
        nc.sync.dma_start(idn[:], idnf_d[:])
        iota = const.tile([P, LC], FP32)
        nc.sync.dma_start(iota[:], iota_d[:])
        onec = const.tile([P, 1], FP32)
        nc.any.memset(onec[:], 1.0)
        bias_b = const.tile([P, DIM], FP32)
        nc.sync.dma_start(bias_b[:], bf_d[:].broadcast_to([P, DIM]))

        cls_f = main.tile([P, DIM], FP32)
        nc.sync.dma_start(cls_f[:], cls_d[:])
        clsT = main.tile([P, EB * P], FP32)
        nc.sync.dma_start(clsT[:], clsT_d[:].rearrange("(e p) b -> p e b", p=P))
        new_f = main.tile([P, DIM], FP32)
        newT = main.tile([P, EB * P], FP32)
        sc4 = main.tile([P, P], FP32)   # rows 0/32/64/96: (t,s) = (0,0) (1,0) (0,1) (1,1)

        KT = wpool.tile([P, EB * DIM], FP32, tag="w0")
        nc.sync.dma_start(KT[:], kt_d[:].rearrange("(e p) d -> p e d", p=P))
        QT = wpool.tile([P, EB * DIM], FP32, tag="w1")
        nc.sync.dma_start(QT[:], qt_d[:].rearrange("(e p) d -> p e d", p=P))
        VT = vpool.tile([P, EB * DIM], FP32)
        nc.sync.dma_start(VT[:], vt_d[:].rearrange("(e p) d -> p e d", p=P))

        # ---- rescore: pick exact winner among LC candidates per row ----
        for m in range(NT):
            cand = rscr.tile([P, CD * LC], FP32, tag="cand")
            nc.sync.dma_start(cand[:],
                              cand_d[:, m * CD * LC:(m + 1) * CD * LC])
            cm = cand[:]
            cm_kl = cm.rearrange("p (k l) -> p k l", l=LC)  # k-major
            sqv = rscr.tile([P, CD * LC], FP32, tag="sq")
            nc.vector.tensor_tensor(out=sqv[:], in0=cm, in1=cm,
                                    op=mybir.AluOpType.mult)
            csq = rscr.tile([P, LC], FP32, tag="csq")
            sq_lk = sqv[:].rearrange("p (k l) -> p l k", l=LC)
            nc.vector.tensor_reduce(out=csq[:], in_=sq_lk,
                                    axis=mybir.AxisListType.X,
                                    op=mybir.AluOpType.add)
            rec = rscr.tile([P, LC], FP32, tag="rec")
            nc.vector.reciprocal(rec[:], csq[:])
            inv = rscr.tile([P, LC], FP32, tag="inv")
            nc.scalar.activation(inv[:], rec[:],
                                 mybir.ActivationFunctionType.Sqrt)
            prod = rscr.tile([P, CD * LC], FP32, tag="prod")
            nc.vector.tensor_tensor(
                out=prod[:].rearrange("p (k l) -> p k l", l=LC),
                in0=cm_kl,
                in1=cls_f[:, m * CD:(m + 1) * CD]
                    .rearrange("p (k o) -> p k o", o=1)
                    .broadcast_to([P, CD, LC]),
                op=mybir.AluOpType.mult)
            dots = rscr.tile([P, LC], FP32, tag="dots")
            pr_lk = prod[:].rearrange("p (k l) -> p l k", l=LC)
            nc.vector.tensor_reduce(out=dots[:], in_=pr_lk,
                                    axis=mybir.AxisListType.X,
                                    op=mybir.AluOpType.add)
            score = rscr.tile([P, LC], FP32, tag="score")
            nc.vector.tensor_tensor(out=score[:], in0=dots[:], in1=inv[:],
                                    op=mybir.AluOpType.mult)
            sv = rscr.tile([P, 8], FP32, tag="sv")
            nc.vector.max(sv[:], score[:])
            su = rscr.tile([P, 8], U32, tag="su")
            nc.vector.max_index(su[:], sv[:], score[:])
            lstar = rscr.tile([P, 1], FP32, tag="lstar")
            nc.vector.tensor_copy(lstar[:], su[:, 0:1])
            oh = rscr.tile([P, LC], FP32, tag="oh")
            nc.vector.tensor_scalar(out=oh[:], in0=iota[:],
                                    scalar1=lstar[:], scalar2=None,
                                    op0=mybir.AluOpType.is_equal)
            sel = rscr.tile([P, CD * LC], FP32, tag="sel")
            nc.vector.tensor_tensor(
                out=sel[:].rearrange("p (k l) -> p k l", l=LC),
                in0=cm_kl,
                in1=oh[:].rearrange("p (o l) -> p o l", o=1)
                    .broadcast_to([P, CD, LC]),
                op=mybir.AluOpType.mult)
            nc.vector.tensor_reduce(
                out=new_f[:, m * CD:(m + 1) * CD],
                in_=sel[:].rearrange("p (k l) -> p k l", l=LC),
                axis=mybir.AxisListType.X, op=mybir.AluOpType.add)

        # ---- newT ----
        for e in range(EB):
            tp = pst.tile([P, P], FP32, tag="tp")
            nc.tensor.transpose(tp[:], new_f[:, e * P:(e + 1) * P], idn[:])
            nc.scalar.copy(newT[:, e * P:(e + 1) * P], tp[:])

        fuseT = [clsT, newT]

        # ---- projections Kt_t, Qt_t once each (cls half overlaps rescore) ----
        kt0 = p2.tile([P, EB * P], FP32, tag="kt0")
        kt1 = p2.tile([P, EB * P], FP32, tag="kt1")
        qt0 = p2.tile([P, EB * P], FP32, tag="qt0")
        qt1 = p2.tile([P, EB * P], FP32, tag="qt1")
        KtS = [kt0, kt1]
        QtS = [qt0, qt1]
        for t in range(2):
            for db in range(EB):
                ps_k = pskq.tile([P, P], FP32, tag="pk")
                for eb in range(EB):
                    nc.tensor.matmul(
                        ps_k[:],
                        lhsT=KT[:, eb * DIM + db * P:eb * DIM + (db + 1) * P],
                        rhs=fuseT[t][:, eb * P:(eb + 1) * P],
                        start=(eb == 0), stop=(eb == EB - 1))
                nc.scalar.copy(KtS[t][:, db * P:(db + 1) * P], ps_k[:])
                ps_q = pskq.tile([P, P], FP32, tag="pq")
                for eb in range(EB):
                    nc.tensor.matmul(
                        ps_q[:],
                        lhsT=QT[:, eb * DIM + db * P:eb * DIM + (db + 1) * P],
                        rhs=fuseT[t][:, eb * P:(eb + 1) * P],
                        start=(eb == 0), stop=(eb == EB - 1))
                nc.scalar.copy(QtS[t][:, db * P:(db + 1) * P], ps_q[:])

        # ---- scores: 4 (t, s) pairs via ones-matmul contraction ----
        for col, (t, s) in enumerate([(0, 0), (1, 0), (0, 1), (1, 1)]):
            ps_sc = pskq.tile([1, P], FP32, tag="psc")
            for db in range(EB):
                prodc = scr.tile([P, P], FP32, tag="prodc")
                nc.vector.tensor_tensor(
                    out=prodc[:], in0=KtS[t][:, db * P:(db + 1) * P],
                    in1=QtS[s][:, db * P:(db + 1) * P],
                    op=mybir.AluOpType.mult)
                nc.tensor.matmul(ps_sc[:], lhsT=onec[:], rhs=prodc[:],
                                 start=(db == 0), stop=(db == EB - 1))
            nc.scalar.copy(sc4[32 * col:32 * col + 1, :], ps_sc[:])

        # Wf halves reuse the KT/QT slots after the score matmuls
        Wf0 = wpool.tile([P, EB * DIM], FP32, tag="w0")
        nc.sync.dma_start(Wf0[:], wft_d[0:DIM, :]
                          .rearrange("(e p) d -> p e d", p=P))
        Wf1 = wpool.tile([P, EB * DIM], FP32, tag="w1")
        nc.sync.dma_start(Wf1[:], wft_d[DIM:2 * DIM, :]
                          .rearrange("(e p) d -> p e d", p=P))
        WfT = [Wf0, Wf1]

        # transpose (128, 128) and pull columns 0/32/64/96 -> (P, 4)
        ps_t = pskq.tile([P, P], FP32, tag="pt4")
        nc.tensor.transpose(ps_t[:], sc4[:], idn[:])
        sc = main.tile([P, 4], FP32)
        sc_src = ps_t[:].rearrange("p (a b) -> p a b", b=32)[:, :, 0:1]
        nc.vector.tensor_copy(sc[:].rearrange("p (a o) -> p a o", o=1), sc_src)

        # softmax over t for each s: cols [ts00, ts10, ts01, ts11]
        ISQ2 = float(1.0 / np.sqrt(np.float32(2.0)))
        Acoef = main.tile([P, 4], FP32)
        for s in range(2):
            mx = scr.tile([P, 1], FP32, tag="mx")
            nc.vector.tensor_tensor(out=mx[:], in0=sc[:, 2 * s:2 * s + 1],
                                    in1=sc[:, 2 * s + 1:2 * s + 2],
                                    op=mybir.AluOpType.max)
            nb = scr.tile([P, 1], FP32, tag="nb")
            nc.vector.tensor_scalar(out=nb[:], in0=mx[:], scalar1=-ISQ2,
                                    scalar2=None, op0=mybir.AluOpType.mult)
            ex = scr.tile([P, 2], FP32, tag="ex")
            nc.scalar.activation(ex[:], sc[:, 2 * s:2 * s + 2],
                                 mybir.ActivationFunctionType.Exp,
                                 bias=nb[:], scale=ISQ2)
            den = scr.tile([P, 1], FP32, tag="den")
            nc.vector.tensor_tensor(out=den[:], in0=ex[:, 0:1], in1=ex[:, 1:2],
                                    op=mybir.AluOpType.add)
            rd = scr.tile([P, 1], FP32, tag="rd")
            nc.vector.reciprocal(rd[:], den[:])
            nc.vector.tensor_scalar(out=Acoef[:, 2 * s:2 * s + 2],
                                    in0=ex[:], scalar1=rd[:], scalar2=None,
                                    op0=mybir.AluOpType.mult)

        # ---- mixes ----
        mixT = []
        for s in range(2):
            t0 = p2s.tile([P, DIM], FP32, tag="m0")
            nc.vector.tensor_scalar(out=t0[:], in0=cls_f[:],
                                    scalar1=Acoef[:, 2 * s:2 * s + 1],
                                    scalar2=None, op0=mybir.AluOpType.mult)
            t1 = p2s.tile([P, DIM], FP32, tag="m1")
            nc.vector.tensor_scalar(out=t1[:], in0=new_f[:],
                                    scalar1=Acoef[:, 2 * s + 1:2 * s + 2],
                                    scalar2=None, op0=mybir.AluOpType.mult)
            mix = p2.tile([P, DIM], FP32, tag=f"mix{s}")
            nc.vector.tensor_tensor(out=mix[:], in0=t0[:], in1=t1[:],
                                    op=mybir.AluOpType.add)
            mT = p2.tile([P, EB * P], FP32, tag=f"kt{s}")
            for e in range(EB):
                tp = pst.tile([P, P], FP32, tag="tp")
                nc.tensor.transpose(tp[:], mix[:, e * P:(e + 1) * P], idn[:])
                nc.scalar.copy(mT[:, e * P:(e + 1) * P], tp[:])
            mixT.append(mT)

        # ---- flatT = [V@mix_0^T ; V@mix_1^T]  (k-chunks, 128 b) ----
        flatT = p2.tile([P, 2 * EB * P], FP32)
        for s in range(2):
            for db in range(EB):
                ps_v = pskq.tile([P, P], FP32, tag="pv")
                for eb in range(EB):
                    nc.tensor.matmul(
                        ps_v[:],
                        lhsT=VT[:, eb * DIM + db * P:eb * DIM + (db + 1) * P],
                        rhs=mixT[s][:, eb * P:(eb + 1) * P],
                        start=(eb == 0), stop=(eb == EB - 1))
                kb = s * EB + db
                nc.scalar.copy(flatT[:, kb * P:(kb + 1) * P], ps_v[:])

        # ---- h = flatT^T @ WfT + bf ----
        hb = p2.tile([P, DIM], FP32, tag="qt0")
        for nbk in range(2):
            ps_h = psh.tile([P, 512], FP32, tag="ph")
            for kb in range(2 * EB):
                wt = WfT[kb // EB]
                ebl = kb % EB
                nc.tensor.matmul(
                    ps_h[:],
                    lhsT=flatT[:, kb * P:(kb + 1) * P],
                    rhs=wt[:, ebl * DIM + nbk * 512:ebl * DIM + (nbk + 1) * 512],
                    start=(kb == 0), stop=(kb == 2 * EB - 1))
            nc.vector.tensor_tensor(out=hb[:, nbk * 512:(nbk + 1) * 512],
                                    in0=ps_h[:],
                                    in1=bias_b[:, nbk * 512:(nbk + 1) * 512],
                                    op=mybir.AluOpType.add)
        sig = p2s.tile([P, DIM], FP32, tag="m0")
        nc.scalar.activation(sig[:], hb[:],
                             mybir.ActivationFunctionType.Sigmoid,
                             scale=1.702)
        hout = p2s.tile([P, DIM], FP32, tag="m1")
        nc.vector.tensor_tensor(out=hout[:], in0=hb[:], in1=sig[:],
                                op=mybir.AluOpType.mult)
        nc.sync.dma_start(h_d[:], hout[:])
    nc.compile()
    return nc


def _get(name, builder):
    if name not in _cache:
        _cache[name] = builder()
    return _cache[name]


def _profile_hook():
    try:
        from trn_agent_boot.trn_boot import _ntff_profile_via_ctypes
        return _ntff_profile_via_ctypes('/opt/axon/libaxon_pjrt.so')
    except Exception:
        return None


def _run_spmd(nc, in_maps, sim=False, tag=""):
    if sim:
        from concourse.bass_interp import CoreSim
        outs = []
        for m in in_maps[:1]:
            cs = CoreSim(nc)
            for k, v in m.items():
                cs.tensor(k)[:] = v
            cs.simulate()
            names = []
            for alloc in nc.m.functions[0].allocations:
                if isinstance(alloc, mybir.MemoryLocationSet) \
                        and alloc.kind == "ExternalOutput":
                    names.append(alloc.memorylocations[0].name)
            outs.append({n: cs.tensor(n).copy() for n in names})
        return outs
    if _PROFILE_DIR:
        hook = _profile_hook()
        if hook is not None:
            out = os.path.join(_PROFILE_DIR, tag)
            os.makedirs(out, exist_ok=True)
            for f in os.listdir(out):
                os.unlink(os.path.join(out, f))
            with hook(out, [0]):
                return run_bass_kernel_spmd(
                    nc, in_maps, list(range(len(in_maps)))).results
    return run_bass_kernel_spmd(nc, in_maps, list(range(len(in_maps)))).results


def kernel(tokens, codebook, K, Q, V, Wf, bf, _sim=False):
    tokens = np.asarray(tokens, np.float32)
    codebook = np.ascontiguousarray(np.asarray(codebook, np.float32))
    K = np.asarray(K, np.float32)
    Q = np.asarray(Q, np.float32)
    V = np.asarray(V, np.float32)
    Wf = np.asarray(Wf, np.float32)
    bf = np.asarray(bf, np.float32)

    bs = tokens.shape[0]
    cls = np.ascontiguousarray(tokens[:, 0, :])          # (1024, 1024)
    idn_bf = np.eye(P, dtype=np.float32)
    import ml_dtypes
    idn_bf16 = idn_bf.astype(ml_dtypes.bfloat16)

    ncores = 1 if _sim else NCORES

    nc_a = _get("a", _build_a)
    in_a = [{"cls": cls[c * P:(c + 1) * P], "cb": codebook, "idn": idn_bf16}
            for c in range(ncores)]
    res_a = _run_spmd(nc_a, in_a, sim=_sim, tag="a")

    # host: expand quad positions -> candidate indices -> gathered vectors
    KT = np.ascontiguousarray(K.T)
    QT = np.ascontiguousarray(Q.T)
    VT = np.ascontiguousarray(V.T)
    WfT = np.ascontiguousarray(Wf.T)
    iota16 = np.broadcast_to(np.arange(LC, dtype=np.float32), (P, LC)).copy()
    idnf = np.eye(P, dtype=np.float32)
    bfv = bf.reshape(1, DIM)

    in_b = []
    for c in range(ncores):
        idx8 = res_a[c]["idx"].reshape(P, NT, 8).astype(np.int64)
        quads = idx8[:, :, :NQ]                              # (P, NT, NQ)
        cj = (quads[..., None] + np.arange(4) * (BOOK // 4)).reshape(P, NT, LC)
        vecs = codebook[cj]                                  # (P, NT, LC, CD)
        cand = np.ascontiguousarray(vecs.transpose(0, 1, 3, 2)).reshape(P, -1)
        in_b.append({
            "cls": cls[c * P:(c + 1) * P],
            "clsT": np.ascontiguousarray(cls[c * P:(c + 1) * P].T),
            "cand": cand,
            "iota": iota16,
            "idnf": idnf,
            "KT": KT, "QT": QT, "VT": VT, "WfT": WfT, "bfv": bfv,
        })

    nc_b = _get("b", _build_b)
    res_b = _run_spmd(nc_b, in_b, sim=_sim, tag="b")

    h = np.concatenate([res_b[c]["h"] for c in range(ncores)], axis=0)
    if _sim:
        return h  # (P, DIM) for one core
    return h.reshape(bs, 1, DIM)

